# revision 1
# baseline (speedup 1.0000x reference)
"""Trainium2 Bass kernel for nn_DeepSymmetricGCN1dBlock.

3-layer GCN block over a shared 2048-node graph, 32 graph copies (b=4, n=8),
channels 128->256->256->256, per-element branch + symmetric max-pooled branch,
training-mode BatchNorm, ReLU.

Strategy
--------
Data-parallel over the 32 graph copies: core k holds copies of graph b=k//2,
n in [4*(k%2), 4*(k%2)+4).  The sparse GCN aggregation is implemented as a
dense matmul against the normalized adjacency A_hat [2048, 2048] (built on
host from edge_index; ~18k nonzeros spread over every 128x128 tile, so dense
tensor-engine matmul beats any gather/scatter scheme).  Per layer:

    agg = A_hat^T-contracted with x    (features-stationary: out is channel-major)
    y   = W^T @ agg                    (channel-major in/out)
    BN stats: per-channel S1/S2 accumulated during PSUM drains,
              AllReduce(add) over all 8 cores
    x' = relu(a1*y1 + a2*y2 + b)       (BN affine fused; relu fused into the
                                        transpose drain back to node-major)

The pooled branch is computed redundantly on both cores of a (b)-pair as a
5th "copy"; the max over n needs one AllReduce(max) over core pairs per layer.
GCN biases are skipped: training-mode BN subtracts the mean, so a per-channel
additive constant cancels exactly.

A_hat is streamed from HBM in 8 chunks of 256 destination columns per layer
(2 MiB each, double buffered) so SBUF holds x (node-major), y (channel-major)
and working tiles in fp32.  All matmuls run as float32r (full-rate fp32).
"""

import sys

if "/opt/trn_rl_repo" not in sys.path:
    sys.path.insert(0, "/opt/trn_rl_repo")

import numpy as np

import concourse.bass as bass
import concourse.bacc as bacc
import concourse.mybir as mybir
import concourse.tile as tile
from concourse.bass_utils import run_bass_kernel_spmd

f32 = mybir.dt.float32
f32r = mybir.dt.float32r
AF = mybir.ActivationFunctionType
OP = mybir.AluOpType

B, N, L, E = 4, 8, 2048, 16384
CH = [128, 256, 256, 256]
EPS = 1e-5
NCORES = 8
GPC = 4            # graph copies per core
LT = L // 128      # 16 node tiles
DQ = 8             # A streamed in DQ chunks of DW destination columns
DW = L // DQ       # 256
CNT_E = 32 * L     # element-branch BN count (all 32 copies)
CNT_P = 8 * L      # pooled-branch BN count (4 graphs, double-counted by pairs)

PAIRS = [[0, 1], [2, 3], [4, 5], [6, 7]]
ALL8 = [list(range(NCORES))]

# Module-level knobs (test.py pokes these)
import os
_PROFILE = False
_SIMULATE = False
_LAYERS = int(os.environ.get("K_LAYERS", "3"))
_NO_CC = os.environ.get("K_NO_CC", "0") == "1"
_CACHE = {}


def _cc(nc, kind, op, groups, bi, bo):
    if _NO_CC:
        nc.sync.dma_start(bo[:], bi[:])
    else:
        nc.gpsimd.collective_compute(kind, op, replica_groups=groups,
                                     ins=[bi[:].opt()], outs=[bo[:].opt()])


def _r(ap):
    return ap.bitcast(f32r)


def _emit(tc, nc, io):
    ctx = tc_ctx = None
    sync, vec, act, gp, te = nc.sync, nc.vector, nc.scalar, nc.gpsimd, nc.tensor

    from contextlib import ExitStack

    ctx = ExitStack()
    with ctx:
        sb = ctx.enter_context(tc.tile_pool(name="sb", bufs=1))
        sb_slot = ctx.enter_context(tc.tile_pool(name="slots", bufs=GPC))
        sb_y1 = ctx.enter_context(tc.tile_pool(name="y1", bufs=GPC + 1))
        sb_ab = ctx.enter_context(tc.tile_pool(name="ab", bufs=2))
        sb_agg = ctx.enter_context(tc.tile_pool(name="agg", bufs=2))
        sb_w = ctx.enter_context(tc.tile_pool(name="w", bufs=4))
        sb_small = ctx.enter_context(tc.tile_pool(name="small", bufs=24))
        ps_a = ctx.enter_context(tc.tile_pool(name="psa", bufs=2, space="PSUM"))
        ps_w = ctx.enter_context(tc.tile_pool(name="psw", bufs=2, space="PSUM"))
        ps_t = ctx.enter_context(tc.tile_pool(name="pst", bufs=2, space="PSUM"))
        dram = ctx.enter_context(tc.tile_pool(name="dram", bufs=1, space="DRAM"))

        # ---- persistent SBUF tiles -------------------------------------
        slots = [sb_slot.tile([128, 4096], f32r, tag="slot", name=f"slot{i}")
                 for i in range(GPC)]
        y1s = [sb_y1.tile([128, 4096], f32, tag="y1", name=f"y1_{i}")
               for i in range(GPC + 1)]
        poolb = sb.tile([128, 4096], f32r, tag="poolb")
        gbt = sb.tile([128, 24], f32, tag="gbt")
        ident = sb.tile([128, 128], f32, tag="ident")
        sqscr = sb.tile([128, DW], f32, tag="sqscr")
        s1pe = sb.tile([128, 2 * GPC * DQ], f32, tag="s1pe")   # [cot][g][dq]
        s1pp = sb.tile([128, 2 * DQ], f32, tag="s1pp")         # [cot][dq]
        s2pe = sb.tile([128, 2 * GPC * DQ], f32, tag="s2pe")   # [cot][g][dq]
        s2pp = sb.tile([128, 2 * DQ], f32, tag="s2pp")         # [cot][dq]
        pack = sb.tile([128, 8], f32, tag="pack")
        glob = sb.tile([128, 8], f32, tag="glob")

        # ---- DRAM bounce tiles for collectives -------------------------
        m1_in = dram.tile([128, 4096], f32, tag="m1i")
        m1_out = dram.tile([128, 4096], f32, tag="m1o")
        m1_in1 = dram.tile([128, 2048], f32, tag="m1i1")
        m1_out1 = dram.tile([128, 2048], f32, tag="m1o1")
        st_in = dram.tile([128, 8], f32, tag="sti")
        st_out = dram.tile([128, 8], f32, tag="sto")

        xsh_d, ash_d, w_d, gb_d, id_d, out_d = (
            io["xsh"], io["Ash"], io["Wmats"], io["gbs"], io["ident"], io["out"])

        sync.dma_start(gbt[:], gb_d[:, :])
        sync.dma_start(ident[:], id_d[:, :])

        # ---- L1 input: load channel-major x, transpose to node-major ----
        for g in range(GPC):
            sync.dma_start(y1s[g][:, 0:L], xsh_d[g, :, :])
        for g in range(GPC):
            for lt in range(LT):
                pt = ps_t.tile([128, 256], f32, tag="pst")
                te.transpose(pt[:, 0:128], y1s[g][:, lt * 128:(lt + 1) * 128], ident[:])
                act.activation(slots[g][:, lt * 128:(lt + 1) * 128], pt[:, 0:128], AF.Copy)

        # ---- L1 pooled input: max over local copies, then pair-max ------
        # (f32 ops into y1s[4] scratch; final ACT copy rounds into f32r poolb)
        mx = y1s[GPC]
        vec.tensor_max(mx[:, 0:L], slots[0][:, 0:L].bitcast(f32),
                      slots[1][:, 0:L].bitcast(f32))
        vec.tensor_max(mx[:, 0:L], mx[:, 0:L], slots[2][:, 0:L].bitcast(f32))
        vec.tensor_max(mx[:, 0:L], mx[:, 0:L], slots[3][:, 0:L].bitcast(f32))
        sync.dma_start(m1_in1[:], mx[:, 0:L])
        _cc(nc, "AllReduce", OP.max, PAIRS, m1_in1, m1_out1)
        sync.dma_start(mx[:, 0:L], m1_out1[:])
        act.activation(poolb[:, 0:L], mx[:, 0:L], AF.Copy)

        for li in range(_LAYERS):
            last = (li == _LAYERS - 1)
            ctn = 1 if li == 0 else 2          # input channel tiles
            cin = 128 * ctn
            we = sb_w.tile([128, 512], f32r, tag="w")
            wp = sb_w.tile([128, 512], f32r, tag="w")
            sync.dma_start(we[:], w_d[li, :, :])
            sync.dma_start(wp[:], w_d[3 + li, :, :])

            # ================= phase 1: matmuls over streamed A ==========
            for dq in range(DQ):
                ab = sb_ab.tile([128, LT * DW], f32r, tag="ab")
                sync.dma_start(ab[:], ash_d[:, dq, :])
                for g in range(GPC + 1):
                    src = slots[g] if g < GPC else poolb
                    dsty = y1s[g]
                    agg = sb_agg.tile([128, 512], f32r, tag="agg")
                    for ct in range(ctn):
                        pa = ps_a.tile([128, DW], f32, tag="psa")
                        for st in range(LT):
                            te.matmul(
                                pa[:],
                                src[:, st * cin + ct * 128: st * cin + ct * 128 + 128],
                                ab[:, st * DW:(st + 1) * DW],
                                start=(st == 0), stop=(st == LT - 1))
                        act.activation(agg[:, ct * DW:(ct + 1) * DW], pa[:], AF.Copy)
                    for cot in range(2):
                        pw = ps_w.tile([128, DW], f32, tag="psw")
                        for ct in range(ctn):
                            te.matmul(
                                pw[:],
                                we[:, ct * 256 + cot * 128: ct * 256 + cot * 128 + 128]
                                if g < GPC else
                                wp[:, ct * 256 + cot * 128: ct * 256 + cot * 128 + 128],
                                agg[:, ct * DW:(ct + 1) * DW],
                                start=(ct == 0), stop=(ct == ctn - 1))
                        dst = dsty[:, cot * 2048 + dq * DW: cot * 2048 + (dq + 1) * DW]
                        if g < GPC:
                            idx = (cot * GPC + g) * DQ + dq
                            s1slot = s1pe[:, idx:idx + 1]
                            s2slot = s2pe[:, idx:idx + 1]
                        else:
                            idx = cot * DQ + dq
                            s1slot = s1pp[:, idx:idx + 1]
                            s2slot = s2pp[:, idx:idx + 1]
                        act.activation(dst, pw[:], AF.Copy, accum_out=s1slot)
                        act.activation(sqscr[:], pw[:], AF.Square, accum_out=s2slot)

            # ================= phase 2: BN stats all-reduce ==============
            for cot in range(2):
                vec.reduce_sum(pack[:, cot:cot + 1],
                               s1pe[:, cot * GPC * DQ:(cot + 1) * GPC * DQ],
                               mybir.AxisListType.X)
                vec.reduce_sum(pack[:, 4 + cot:5 + cot],
                               s1pp[:, cot * DQ:(cot + 1) * DQ],
                               mybir.AxisListType.X)
            for cot in range(2):
                vec.reduce_sum(pack[:, 2 + cot:3 + cot],
                               s2pe[:, cot * GPC * DQ:(cot + 1) * GPC * DQ],
                               mybir.AxisListType.X)
                vec.reduce_sum(pack[:, 6 + cot:7 + cot],
                               s2pp[:, cot * DQ:(cot + 1) * DQ],
                               mybir.AxisListType.X)
            sync.dma_start(st_in[:], pack[:])
            _cc(nc, "AllReduce", OP.add, ALL8, st_in, st_out)
            sync.dma_start(glob[:], st_out[:])

            # affine coefficients: a = g*rsqrt(var+eps), b = be - a*mean
            eps_t = sb_small.tile([128, 1], f32, tag="sm")
            vec.memset(eps_t[:], EPS)
            me = sb_small.tile([128, 2], f32, tag="sm")
            ve = sb_small.tile([128, 2], f32, tag="sm")
            t0 = sb_small.tile([128, 2], f32, tag="sm")
            a1 = sb_small.tile([128, 2], f32, tag="sm")
            b1 = sb_small.tile([128, 2], f32, tag="sm")
            mp = sb_small.tile([128, 2], f32, tag="sm")
            vp = sb_small.tile([128, 2], f32, tag="sm")
            a2 = sb_small.tile([128, 2], f32, tag="sm")
            bs = sb_small.tile([128, 2], f32, tag="sm")

            def affine(a_t, b_t, m_t, v_t, s1_ap, s2_ap, inv_cnt, gslc, beslc):
                vec.tensor_scalar(m_t[:], s1_ap, inv_cnt, None, OP.mult)
                vec.tensor_scalar(v_t[:], s2_ap, inv_cnt, None, OP.mult)
                vec.tensor_tensor(t0[:], m_t[:], m_t[:], OP.mult)
                vec.tensor_tensor(v_t[:], v_t[:], t0[:], OP.subtract)
                act.activation(t0[:], v_t[:], AF.Sqrt, bias=eps_t[:])
                vec.reciprocal(t0[:], t0[:])
                vec.tensor_tensor(a_t[:], gslc, t0[:], OP.mult)
                vec.tensor_tensor(t0[:], a_t[:], m_t[:], OP.mult)
                vec.tensor_tensor(b_t[:], beslc, t0[:], OP.subtract)

            affine(a1, b1, me, ve, glob[:, 0:2], glob[:, 2:4], 1.0 / CNT_E,
                   gbt[:, 4 * li: 4 * li + 2], gbt[:, 4 * li + 2: 4 * li + 4])
            affine(a2, bs, mp, vp, glob[:, 4:6], glob[:, 6:8], 1.0 / CNT_P,
                   gbt[:, 12 + 4 * li: 14 + 4 * li], gbt[:, 14 + 4 * li: 16 + 4 * li])
            vec.tensor_tensor(bs[:], b1[:], bs[:], OP.add)  # b1+b2 combined

            # y2sb = a2*y2 + (b1+b2), in place on the pooled y
            for cot in range(2):
                vec.tensor_scalar(y1s[GPC][:, cot * 2048:(cot + 1) * 2048],
                                  y1s[GPC][:, cot * 2048:(cot + 1) * 2048],
                                  a2[:, cot:cot + 1], bs[:, cot:cot + 1],
                                  OP.mult, OP.add)

            # ================= phase 3: x' = relu(a1*y1 + y2sb) ==========
            for g in range(GPC):
                for cot in range(2):
                    vec.scalar_tensor_tensor(
                        y1s[g][:, cot * 2048:(cot + 1) * 2048],
                        y1s[g][:, cot * 2048:(cot + 1) * 2048],
                        a1[:, cot:cot + 1],
                        y1s[GPC][:, cot * 2048:(cot + 1) * 2048],
                        OP.mult, OP.add)
                if not last:
                    # transpose back to node-major with fused relu
                    for lt in range(LT):
                        pt = ps_t.tile([128, 256], f32, tag="pst")
                        te.transpose(pt[:, 0:128],
                                     y1s[g][:, lt * 128:(lt + 1) * 128], ident[:])
                        te.transpose(pt[:, 128:256],
                                     y1s[g][:, 2048 + lt * 128: 2048 + (lt + 1) * 128],
                                     ident[:])
                        act.activation(slots[g][:, lt * 256:(lt + 1) * 256],
                                       pt[:], AF.Relu)
                else:
                    for cot in range(2):
                        act.activation(y1s[g][:, cot * 2048:(cot + 1) * 2048],
                                       y1s[g][:, cot * 2048:(cot + 1) * 2048],
                                       AF.Relu)
                        sync.dma_start(out_d[g, cot * 128:(cot + 1) * 128, :],
                                       y1s[g][:, cot * 2048:(cot + 1) * 2048])

            if not last:
                # pooled input for the next layer: local max then pair-max
                mx = y1s[GPC]
                vec.tensor_max(mx[:], slots[0][:].bitcast(f32),
                              slots[1][:].bitcast(f32))
                vec.tensor_max(mx[:], mx[:], slots[2][:].bitcast(f32))
                vec.tensor_max(mx[:], mx[:], slots[3][:].bitcast(f32))
                sync.dma_start(m1_in[:], mx[:])
                _cc(nc, "AllReduce", OP.max, PAIRS, m1_in, m1_out)
                sync.dma_start(mx[:], m1_out[:])
                act.activation(poolb[:], mx[:], AF.Copy)


def _build():
    key = ("nc", _LAYERS, _NO_CC)
    if key in _CACHE:
        return _CACHE[key]
    nc = bacc.Bacc("TRN2", target_bir_lowering=False, debug=False,
                   num_devices=NCORES)
    io = {
        "xsh": nc.dram_tensor("xsh", [GPC, 128, L], f32, kind="ExternalInput"),
        "Ash": nc.dram_tensor("Ash", [128, DQ, LT * DW], f32r, kind="ExternalInput"),
        "Wmats": nc.dram_tensor("Wmats", [6, 128, 512], f32r, kind="ExternalInput"),
        "gbs": nc.dram_tensor("gbs", [128, 24], f32, kind="ExternalInput"),
        "ident": nc.dram_tensor("ident", [128, 128], f32, kind="ExternalInput"),
        "out": nc.dram_tensor("out", [GPC, 256, L], f32, kind="ExternalOutput"),
    }
    with tile.TileContext(nc) as tc:
        _emit(tc, nc, io)
    nc.compile()
    _CACHE[key] = nc
    return nc


def _tf32(a):
    """Round f32 to TF32 (10-bit mantissa, RNE) — fp32r's precision."""
    u = np.ascontiguousarray(a, np.float32).view(np.uint32)
    r = (u + np.uint32(0xFFF) + ((u >> np.uint32(13)) & np.uint32(1))) & np.uint32(0xFFFFE000)
    return r.view(np.float32)


def _host_prep(x, edge_index, Ws, gs, bes):
    """Build the device-layout arrays on host."""
    src = np.asarray(edge_index[0], dtype=np.int64)
    dst = np.asarray(edge_index[1], dtype=np.int64)
    deg = np.zeros(L, np.float32)
    np.add.at(deg, dst, np.float32(1.0))
    deg += np.float32(2.0)
    dis = (1.0 / np.sqrt(deg.astype(np.float64))).astype(np.float32)
    A = np.zeros((L, L), np.float32)
    np.add.at(A, (src, dst), dis[src] * dis[dst])
    A[np.arange(L), np.arange(L)] += np.float32(2.0) * dis * dis
    ash = _tf32(np.ascontiguousarray(
        A.reshape(LT, 128, DQ, DW).transpose(1, 2, 0, 3).reshape(128, DQ, LT * DW)))

    wm = np.zeros((6, 128, 512), np.float32)
    for i, W in enumerate(Ws):
        cin = W.shape[0]
        wm[i, :, : (cin // 128) * 256] = np.ascontiguousarray(
            W.reshape(cin // 128, 128, 256).transpose(1, 0, 2).reshape(128, -1))
    wm = _tf32(wm)

    gb = np.zeros((128, 24), np.float32)
    vecs = [gs[0], bes[0], gs[1], bes[1], gs[2], bes[2],
            gs[3], bes[3], gs[4], bes[4], gs[5], bes[5]]
    for v, w in enumerate(vecs):
        gb[:, v * 2 + 0] = w[0:128]
        gb[:, v * 2 + 1] = w[128:256]

    ident = np.eye(128, dtype=np.float32)
    return ash, wm, gb, ident


def kernel(x, edge_index, W1, b1, W2, b2, W3, b3, W1s, b1s, W2s, b2s, W3s, b3s,
           g1, be1, g2, be2, g3, be3, g1s, be1s, g2s, be2s, g3s, be3s):
    x = np.asarray(x, np.float32)
    ash, wm, gb, ident = _host_prep(
        x, np.asarray(edge_index),
        [np.asarray(W1, np.float32), np.asarray(W2, np.float32),
         np.asarray(W3, np.float32), np.asarray(W1s, np.float32),
         np.asarray(W2s, np.float32), np.asarray(W3s, np.float32)],
        [np.asarray(g1, np.float32), np.asarray(g2, np.float32),
         np.asarray(g3, np.float32), np.asarray(g1s, np.float32),
         np.asarray(g2s, np.float32), np.asarray(g3s, np.float32)],
        [np.asarray(be1, np.float32), np.asarray(be2, np.float32),
         np.asarray(be3, np.float32), np.asarray(be1s, np.float32),
         np.asarray(be2s, np.float32), np.asarray(be3s, np.float32)])

    # core k: graph b=k//2, copies n in [4*(k%2), 4*(k%2)+4)
    xr = x.reshape(B * N, CH[0], L).reshape(NCORES, GPC, CH[0], L)
    in_maps = []
    for k in range(NCORES):
        in_maps.append({
            "xsh": np.ascontiguousarray(xr[k]),
            "Ash": ash, "Wmats": wm, "gbs": gb, "ident": ident,
        })

    nc = _build()

    if _SIMULATE:
        from concourse.bass_interp import MultiCoreSim
        sim = MultiCoreSim(nc, NCORES)
        for k in range(NCORES):
            for nm, arr in in_maps[k].items():
                sim.cores[k].tensor(nm)[:] = arr
        sim.simulate(check_with_hw=False)
        outs = [np.array(sim.cores[k].mem_tensor("out")).reshape(GPC, 256, L)
                for k in range(NCORES)]
        return np.concatenate(outs, axis=0)

    res = run_bass_kernel_spmd(nc, in_maps, core_ids=list(range(NCORES)),
                               trace=_PROFILE)
    if _PROFILE:
        _CACHE["last_result"] = res
    outs = [np.asarray(res.results[k]["out"]).reshape(GPC, 256, L)
            for k in range(NCORES)]
    return np.concatenate(outs, axis=0).astype(np.float32)



# revision 11
# speedup vs baseline: 1.2862x; 1.2862x over previous
"""Trainium2 Bass kernel for nn_DeepSymmetricGCN1dBlock.

3-layer GCN block over a shared 2048-node graph, 32 graph copies (b=4, n=8),
channels 128->256->256->256, per-element branch + symmetric max-pooled branch,
training-mode BatchNorm, ReLU.

Strategy (v3)
-------------
Data-parallel over the 32 graph copies: core k holds copies of graph b=k//2,
n in [4*(k%2), 4*(k%2)+4).  The sparse GCN aggregation is a dense matmul
against the normalized adjacency A_hat [2048, 2048] (built on host), held
RESIDENT in SBUF in fp16 (8 MiB) for all three layers.

All compute is fp16 into f32 PSUM.  Zero on-device transposes:

  L1 (A-then-W): x arrives node-major from the host (pre-transposed);
      agg[c,dst] = sum_src x[src,c] * A[src,dst]   (x slices stationary)
      y[cout,dst] = W^T contracted with agg        (W slices stationary)
      -> Y channel-major.
  L2/L3 (W-then-A): input x' channel-major (= previous Y);
      h[node,cout] = sum_cin x'[cin,node] * W[cin,cout]  (x' slices stationary)
      y[cout,dst]  = sum_src h[src,cout] * A[src,dst]    (h slices stationary)
      -> Y channel-major again: the cycle closes with no transposes.

The L1 pooled input (max over all n of the raw x) is computed on HOST, which
removes the initial pair-collective entirely.  BN statistics are computed
with chunked vector-engine bn_stats fired right after each PSUM drain (so
only ~2us of stats work remains after the last matmul of a layer), then
bn_aggr + a tiny conversion to (S1, S2) sums for the cross-core AllReduce.
Copy order per layer is g0,g1,g2,pooled,g3 so the 1 MiB pair AllReduce(max)
of the pooled input has ~90us of element-copy compute to hide behind, and
the pooled-stats AllReduce hides behind g3.  Phase 3 (x' = relu(a1*y1 +
y2sb)) is chunked [128,512] so the next layer's W-step starts ~2us after
the affine coefficients land (sub-tile dependency tracking).

GCN biases are skipped: training-mode BN subtracts the mean, so a
per-channel additive constant cancels exactly.
"""

import sys

if "/opt/trn_rl_repo" not in sys.path:
    sys.path.insert(0, "/opt/trn_rl_repo")

import numpy as np

import concourse.bass as bass
import concourse.bacc as bacc
import concourse.mybir as mybir
import concourse.tile as tile
from concourse.bass_utils import run_bass_kernel_spmd

f32 = mybir.dt.float32
fp16 = mybir.dt.float16
AF = mybir.ActivationFunctionType
OP = mybir.AluOpType
AX = mybir.AxisListType

B, N, L, E = 4, 8, 2048, 16384
CH = [128, 256, 256, 256]
EPS = 1e-5
NCORES = 8
GPC = 4            # graph copies per core
LT = L // 128      # 16 node tiles
CNT_E = 32 * L     # element-branch BN count (all 32 copies)
CNT_P = 8 * L      # pooled-branch BN count (4 graphs, double-counted by pairs)

PAIRS = [[0, 1], [2, 3], [4, 5], [6, 7]]
ALL8 = [list(range(NCORES))]

# Module-level knobs (test.py pokes these)
import os
_PROFILE = False
_SIMULATE = False
_LAYERS = int(os.environ.get("K_LAYERS", "3"))
_NO_CC = os.environ.get("K_NO_CC", "0") == "1"
_CACHE = {}


def _cc(nc, kind, op, groups, bi, bo):
    if _NO_CC:
        nc.sync.dma_start(bo[:], bi[:])
    else:
        nc.gpsimd.collective_compute(kind, op, replica_groups=groups,
                                     ins=[bi[:].opt()], outs=[bo[:].opt()])


def _emit(tc, nc, io):
    sync, vec, act, gp, te = nc.sync, nc.vector, nc.scalar, nc.gpsimd, nc.tensor

    from contextlib import ExitStack

    ctx = ExitStack()
    with ctx:
        sb = ctx.enter_context(tc.tile_pool(name="sb", bufs=1))
        sb_a = ctx.enter_context(tc.tile_pool(name="a", bufs=4))
        sb_x = ctx.enter_context(tc.tile_pool(name="x", bufs=5))
        sb_h = ctx.enter_context(tc.tile_pool(name="h", bufs=2))
        sb_small = ctx.enter_context(tc.tile_pool(name="small", bufs=28))
        ps_a = ctx.enter_context(tc.tile_pool(name="psa", bufs=6, space="PSUM"))
        ps_w = ctx.enter_context(tc.tile_pool(name="psw", bufs=2, space="PSUM"))
        dram = ctx.enter_context(tc.tile_pool(name="dram", bufs=1, space="DRAM"))

        # ---- persistent SBUF tiles -------------------------------------
        # y1s[0..3]: element-copy Y / x' (channel-major, [cot*2048 + node]);
        # y1s[4]: pooled Y / y2sb scratch
        y1s = [sb.tile([128, 4096], fp16, tag=f"y1_{i}", name=f"y1_{i}")
               for i in range(GPC + 1)]
        poolb = sb.tile([128, 4096], fp16, tag="poolb")   # pooled x' (L2/L3 in)
        wall = sb.tile([128, 6 * 512], fp16, tag="wall")  # all W mats
        gbt = sb.tile([128, 24], f32, tag="gbt")
        # A resident: 4 tiles of 4 source-tile groups: [p, (st%4)*2048 + dst]
        ares = [sb_a.tile([128, 4 * L], fp16, tag="ares", name=f"ares{i}")
                for i in range(4)]
        # bn_stats strips: elem [cot][g][chunk] 6-tuples; pooled [cot][chunk]
        bnpe = sb.tile([128, 2 * GPC * 4 * 6], f32, tag="bnpe")   # 192 cols
        bnpp = sb.tile([128, 2 * 4 * 6], f32, tag="bnpp")         # 48 cols
        sete = sb.tile([128, 4], f32, tag="sete")   # S1c0,S1c1,S2c0,S2c1
        setp = sb.tile([128, 4], f32, tag="setp")
        globe = sb.tile([128, 4], f32, tag="globe")
        globp = sb.tile([128, 4], f32, tag="globp")

        # ---- DRAM bounce tiles for collectives -------------------------
        m_in = dram.tile([128, 4096], fp16, tag="mi")
        m_out = dram.tile([128, 4096], fp16, tag="mo")
        se_in = dram.tile([128, 4], f32, tag="sei")
        se_out = dram.tile([128, 4], f32, tag="seo")
        sp_in = dram.tile([128, 4], f32, tag="spi")
        sp_out = dram.tile([128, 4], f32, tag="spo")

        xnm_d, pb1_d, a_d, w_d, gb_d, out_d = (
            io["xnm"], io["pb1"], io["Ares"], io["Wall"], io["gbs"], io["out"])

        # ---- initial DMAs (A group 0 + first copy first) ---------------
        sync.dma_start(ares[0][:], a_d[:, 0:4 * L])
        xts = []
        for g in range(GPC):
            xt = sb_x.tile([128, 2048], fp16, tag="xnm", name=f"xnm{g}")
            sync.dma_start(xt[:], xnm_d[g, :, :])
            xts.append(xt)
        for i in range(1, 4):
            sync.dma_start(ares[i][:], a_d[:, i * 4 * L:(i + 1) * 4 * L])
        pb1 = sb_x.tile([128, 2048], fp16, tag="xnm", name="pb1")
        sync.dma_start(pb1[:], pb1_d[:, :])
        sync.dma_start(wall[:], w_d[:, :])
        sync.dma_start(gbt[:], gb_d[:, :])

        def wslice(li, pooled, ct):
            idx = (3 + li) if pooled else li
            return wall[:, idx * 512 + ct * 256: idx * 512 + (ct + 1) * 256]

        def a_ap(st, c0, c1):
            return ares[st // 4][:, (st % 4) * L + c0:(st % 4) * L + c1]

        def bn_chunk(g, cot, ch, src_ap):
            """bn_stats of one [128,512] drained chunk into its strip slot."""
            if g == GPC:
                col = (cot * 4 + ch) * 6
                vec.bn_stats(bnpp[:, col:col + 6], src_ap)
            else:
                col = ((cot * GPC + g) * 4 + ch) * 6
                vec.bn_stats(bnpe[:, col:col + 6], src_ap)

        # ---------------------------------------------------------------
        def l1_copy(g, src_nm, pooled):
            """L1 A-then-W for one copy. src_nm: [128, 2048] node-major."""
            ydst = y1s[g]
            # A-step: agg[cin=128, dst] ; 4 psum chunks of 512 dst
            pas = [ps_a.tile([128, 512], f32, tag="psa", name=f"pa{i}")
                   for i in range(4)]
            for st in range(LT):
                lh = src_nm[:, st * 128:(st + 1) * 128]
                for dqh in range(4):
                    te.matmul(pas[dqh][:], lh,
                              a_ap(st, dqh * 512, (dqh + 1) * 512),
                              start=(st == 0), stop=(st == LT - 1))
            agg = sb_h.tile([128, 2048], fp16, tag="agg")
            for dqh in range(4):
                act.activation(agg[:, dqh * 512:(dqh + 1) * 512],
                               pas[dqh][:], AF.Copy)
            # W-step: y[cout_half, 512 dst] per (cot, dch)
            for cot in range(2):
                wsl = wslice(0, pooled, 0)[:, cot * 128:(cot + 1) * 128]
                for dch in range(4):
                    py = ps_a.tile([128, 512], f32, tag="psa", name="py")
                    te.matmul(py[:], wsl, agg[:, dch * 512:(dch + 1) * 512])
                    dst = ydst[:, cot * 2048 + dch * 512: cot * 2048 + (dch + 1) * 512]
                    act.activation(dst, py[:], AF.Copy)
                    bn_chunk(g, cot, dch, dst)

        def lx_copy(li, g, src_cm, pooled):
            """L2/L3 W-then-A for one copy. src_cm: [128, 4096] channel-major."""
            ydst = y1s[g]
            # W-step: h[node, cout] per node tile
            h = sb_h.tile([128, 4096], fp16, tag="h")
            for nt in range(LT):
                ph = ps_w.tile([128, 256], f32, tag="psw", name="ph")
                for ct in range(2):
                    te.matmul(ph[:],
                              src_cm[:, ct * 2048 + nt * 128: ct * 2048 + (nt + 1) * 128],
                              wslice(li, pooled, ct),
                              start=(ct == 0), stop=(ct == 1))
                act.activation(h[:, nt * 256:(nt + 1) * 256], ph[:], AF.Copy)
            # A-step: y[cout_half, dst] accumulated over source tiles
            for cot in range(2):
                pas = [ps_a.tile([128, 512], f32, tag="psa", name=f"pb{i}")
                       for i in range(4)]
                for st in range(LT):
                    lh = h[:, st * 256 + cot * 128: st * 256 + cot * 128 + 128]
                    for dqh in range(4):
                        te.matmul(pas[dqh][:], lh,
                                  a_ap(st, dqh * 512, (dqh + 1) * 512),
                                  start=(st == 0), stop=(st == LT - 1))
                for dqh in range(4):
                    dst = ydst[:, cot * 2048 + dqh * 512: cot * 2048 + (dqh + 1) * 512]
                    act.activation(dst, pas[dqh][:], AF.Copy)
                    bn_chunk(g, cot, dqh, dst)

        # ---- stats aggregation + affine --------------------------------
        eps_t = sb_small.tile([128, 1], f32, tag="sm")
        vec.memset(eps_t[:], EPS)
        t0 = sb_small.tile([128, 2], f32, tag="sm")
        t1 = sb_small.tile([128, 1], f32, tag="sm")
        a1 = sb_small.tile([128, 2], f32, tag="sm")
        b1 = sb_small.tile([128, 2], f32, tag="sm")
        a2 = sb_small.tile([128, 2], f32, tag="sm")
        bs = sb_small.tile([128, 2], f32, tag="sm")
        me = sb_small.tile([128, 2], f32, tag="sm")
        ve = sb_small.tile([128, 2], f32, tag="sm")
        eagg = sb_small.tile([128, 4], f32, tag="sm")   # (mean,var) per cot
        pagg = sb_small.tile([128, 4], f32, tag="sm")

        def fold_stats(agg_t, bnp, ncols, set_t, cnt):
            """bn_aggr per cot + convert to (S1, S2) sums in set_t."""
            for cot in range(2):
                vec.bn_aggr(agg_t[:, 2 * cot:2 * cot + 2],
                            bnp[:, cot * ncols:(cot + 1) * ncols])
                # S1 = mean*cnt ; S2 = (var + mean^2)*cnt
                vec.tensor_scalar(set_t[:, cot:cot + 1],
                                  agg_t[:, 2 * cot:2 * cot + 1], float(cnt),
                                  None, OP.mult)
                vec.tensor_tensor(t1[:], agg_t[:, 2 * cot:2 * cot + 1],
                                  agg_t[:, 2 * cot:2 * cot + 1], OP.mult)
                vec.tensor_tensor(t1[:], agg_t[:, 2 * cot + 1:2 * cot + 2],
                                  t1[:], OP.add)
                vec.tensor_scalar(set_t[:, 2 + cot:3 + cot], t1[:], float(cnt),
                                  None, OP.mult)

        def affine(a_t, b_t, s1_ap, s2_ap, inv_cnt, gslc, beslc):
            vec.tensor_scalar(me[:], s1_ap, inv_cnt, None, OP.mult)
            vec.tensor_scalar(ve[:], s2_ap, inv_cnt, None, OP.mult)
            vec.tensor_tensor(t0[:], me[:], me[:], OP.mult)
            vec.tensor_tensor(ve[:], ve[:], t0[:], OP.subtract)
            act.activation(t0[:], ve[:], AF.Sqrt, bias=eps_t[:])
            vec.reciprocal(t0[:], t0[:])
            vec.tensor_tensor(a_t[:], gslc, t0[:], OP.mult)
            vec.tensor_tensor(t0[:], a_t[:], me[:], OP.mult)
            vec.tensor_tensor(b_t[:], beslc, t0[:], OP.subtract)

        def elem_stats_ar():
            fold_stats(eagg, bnpe, GPC * 4 * 6, sete, GPC * L)
            sync.dma_start(se_in[:], sete[:])
            _cc(nc, "AllReduce", OP.add, ALL8, se_in, se_out)
            sync.dma_start(globe[:], se_out[:])

        def pool_stats_ar():
            fold_stats(pagg, bnpp, 4 * 6, setp, L)
            sync.dma_start(sp_in[:], setp[:])
            _cc(nc, "AllReduce", OP.add, ALL8, sp_in, sp_out)
            sync.dma_start(globp[:], sp_out[:])

        # ---- the three layers ------------------------------------------
        for li in range(_LAYERS):
            last = (li == _LAYERS - 1)
            if li == 0:
                for g in range(GPC - 1):
                    l1_copy(g, xts[g], False)
                l1_copy(GPC, pb1, True)
                pool_stats_ar()
                l1_copy(GPC - 1, xts[GPC - 1], False)
                elem_stats_ar()
            else:
                src = [y1s[g] for g in range(GPC)]
                # order: g0, g1, g2, pooled, g3
                for g in range(GPC - 1):
                    lx_copy(li, g, src[g], False)
                lx_copy(li, GPC, poolb, True)
                pool_stats_ar()
                lx_copy(li, GPC - 1, src[GPC - 1], False)
                elem_stats_ar()

            # BN affine coefficients
            affine(a1, b1, globe[:, 0:2], globe[:, 2:4], 1.0 / CNT_E,
                   gbt[:, 4 * li: 4 * li + 2], gbt[:, 4 * li + 2: 4 * li + 4])
            affine(a2, bs, globp[:, 0:2], globp[:, 2:4], 1.0 / CNT_P,
                   gbt[:, 12 + 4 * li: 14 + 4 * li], gbt[:, 14 + 4 * li: 16 + 4 * li])
            vec.tensor_tensor(bs[:], b1[:], bs[:], OP.add)  # b1+b2 combined

            # phase 3: x' = relu(a1*y1 + y2sb), chunked [128,512] so the
            # next layer's W-step can start after g0's first chunks.
            # y2sb = a2*y2 + (b1+b2) in place on the pooled y, also chunked.
            def p3_chunk(g, cot, ch):
                c0 = cot * 2048 + ch * 512
                vec.scalar_tensor_tensor(
                    y1s[g][:, c0:c0 + 512], y1s[g][:, c0:c0 + 512],
                    a1[:, cot:cot + 1], y1s[GPC][:, c0:c0 + 512],
                    OP.mult, OP.add)
                act.activation(y1s[g][:, c0:c0 + 512],
                               y1s[g][:, c0:c0 + 512], AF.Relu)
                if last:
                    sync.dma_start(
                        out_d[g, cot * 128:(cot + 1) * 128,
                              ch * 512:(ch + 1) * 512],
                        y1s[g][:, c0:c0 + 512])

            for ch in range(4):
                for cot in range(2):
                    c0 = cot * 2048 + ch * 512
                    vec.tensor_scalar(y1s[GPC][:, c0:c0 + 512],
                                      y1s[GPC][:, c0:c0 + 512],
                                      a2[:, cot:cot + 1], bs[:, cot:cot + 1],
                                      OP.mult, OP.add)
                for cot in range(2):
                    p3_chunk(0, cot, ch)
            for g in range(1, GPC):
                for ch in range(4):
                    for cot in range(2):
                        p3_chunk(g, cot, ch)

            if not last:
                # pooled input for the next layer: local max then pair-max
                mx = sb_h.tile([128, 4096], fp16, tag="h")
                vec.tensor_max(mx[:], y1s[0][:], y1s[1][:])
                vec.tensor_max(mx[:], mx[:], y1s[2][:])
                vec.tensor_max(mx[:], mx[:], y1s[3][:])
                sync.dma_start(m_in[:], mx[:])
                _cc(nc, "AllReduce", OP.max, PAIRS, m_in, m_out)
                sync.dma_start(poolb[:], m_out[:])


def _build():
    key = ("nc", _LAYERS, _NO_CC)
    if key in _CACHE:
        return _CACHE[key]
    nc = bacc.Bacc("TRN2", target_bir_lowering=False, debug=False,
                   num_devices=NCORES)
    io = {
        "xnm": nc.dram_tensor("xnm", [GPC, 128, L], fp16, kind="ExternalInput"),
        "pb1": nc.dram_tensor("pb1", [128, L], fp16, kind="ExternalInput"),
        "Ares": nc.dram_tensor("Ares", [128, LT * L], fp16, kind="ExternalInput"),
        "Wall": nc.dram_tensor("Wall", [128, 6 * 512], fp16, kind="ExternalInput"),
        "gbs": nc.dram_tensor("gbs", [128, 24], f32, kind="ExternalInput"),
        "out": nc.dram_tensor("out", [GPC, 256, L], fp16, kind="ExternalOutput"),
    }
    with tile.TileContext(nc) as tc:
        _emit(tc, nc, io)
    nc.compile()
    _CACHE[key] = nc
    return nc


def _host_prep(edge_index, Ws, gs, bes):
    """Build the device-layout arrays on host."""
    src = np.asarray(edge_index[0], dtype=np.int64)
    dst = np.asarray(edge_index[1], dtype=np.int64)
    deg = np.zeros(L, np.float32)
    np.add.at(deg, dst, np.float32(1.0))
    deg += np.float32(2.0)
    dis = (1.0 / np.sqrt(deg.astype(np.float64))).astype(np.float32)
    A = np.zeros((L, L), np.float32)
    np.add.at(A, (src, dst), dis[src] * dis[dst])
    A[np.arange(L), np.arange(L)] += np.float32(2.0) * dis * dis
    # resident layout: [p, st*L + dst] with src = st*128 + p
    ares = np.ascontiguousarray(
        A.reshape(LT, 128, L).transpose(1, 0, 2).reshape(128, LT * L)
    ).astype(np.float16)

    wall = np.zeros((128, 6 * 512), np.float32)
    for i, W in enumerate(Ws):
        cin = W.shape[0]
        for ct in range(cin // 128):
            wall[:, i * 512 + ct * 256:i * 512 + (ct + 1) * 256] = \
                W[ct * 128:(ct + 1) * 128, :]
    wall = wall.astype(np.float16)

    gb = np.zeros((128, 24), np.float32)
    vecs = [gs[0], bes[0], gs[1], bes[1], gs[2], bes[2],
            gs[3], bes[3], gs[4], bes[4], gs[5], bes[5]]
    for v, w in enumerate(vecs):
        gb[:, v * 2 + 0] = w[0:128]
        gb[:, v * 2 + 1] = w[128:256]
    return ares, wall, gb


def _to_nm(xc):
    """[128, L] channel-major f32 -> [128, L] node-major fp16 host layout.

    node-major tile: partitions p = node within tile, cols = nt*128 + c
    with node = nt*128 + p."""
    t = xc.T.reshape(LT, 128, 128).transpose(1, 0, 2).reshape(128, L)
    return np.ascontiguousarray(t).astype(np.float16)


def kernel(x, edge_index, W1, b1, W2, b2, W3, b3, W1s, b1s, W2s, b2s, W3s, b3s,
           g1, be1, g2, be2, g3, be3, g1s, be1s, g2s, be2s, g3s, be3s):
    x = np.asarray(x, np.float32)
    ares, wall, gb = _host_prep(
        np.asarray(edge_index),
        [np.asarray(W1, np.float32), np.asarray(W2, np.float32),
         np.asarray(W3, np.float32), np.asarray(W1s, np.float32),
         np.asarray(W2s, np.float32), np.asarray(W3s, np.float32)],
        [np.asarray(g1, np.float32), np.asarray(g2, np.float32),
         np.asarray(g3, np.float32), np.asarray(g1s, np.float32),
         np.asarray(g2s, np.float32), np.asarray(g3s, np.float32)],
        [np.asarray(be1, np.float32), np.asarray(be2, np.float32),
         np.asarray(be3, np.float32), np.asarray(be1s, np.float32),
         np.asarray(be2s, np.float32), np.asarray(be3s, np.float32)])

    # core k: graph b=k//2, copies n in [4*(k%2), 4*(k%2)+4)
    xr = x.reshape(B * N, CH[0], L).reshape(NCORES, GPC, CH[0], L)
    xp = x.max(axis=1)  # [B, 128, L] pooled L1 input
    in_maps = []
    for k in range(NCORES):
        xnm = np.stack([_to_nm(xr[k, g]) for g in range(GPC)])
        in_maps.append({
            "xnm": xnm,
            "pb1": _to_nm(xp[k // 2]),
            "Ares": ares, "Wall": wall, "gbs": gb,
        })

    nc = _build()

    if _SIMULATE:
        from concourse.bass_interp import MultiCoreSim
        sim = MultiCoreSim(nc, NCORES)
        for k in range(NCORES):
            for nm, arr in in_maps[k].items():
                sim.cores[k].tensor(nm)[:] = arr
        sim.simulate(check_with_hw=False)
        outs = [np.array(sim.cores[k].mem_tensor("out")).reshape(GPC, 256, L)
                for k in range(NCORES)]
        return np.concatenate(outs, axis=0).astype(np.float32)

    res = run_bass_kernel_spmd(nc, in_maps, core_ids=list(range(NCORES)),
                               trace=_PROFILE)
    if _PROFILE:
        _CACHE["last_result"] = res
    outs = [np.asarray(res.results[k]["out"]).reshape(GPC, 256, L)
            for k in range(NCORES)]
    return np.concatenate(outs, axis=0).astype(np.float32)
